# revision 33
# baseline (speedup 1.0000x reference)
"""BiRWKV layer kernel for 8 Trainium2 NeuronCores.

Strategy (data-parallel over B=8, one batch element per core):
  - (channel, time) layout on chip: channels on the 128 SBUF partitions
    (C=512 -> 4 blocks), time on the free dim.
  - r/k/v projections for both directions are bf16 matmuls
    (lhsT = W block, rhs = x^T block) accumulated over 4 input-channel
    blocks into PSUM (fp32).
  - WKV runs UNSTABILIZED (mathematically equal to the reference's
    log-sum-exp form; values stay in range since |w|*T <= ~28, k~N(0,1)):
        den_t = d*den_{t-1} + e^{k_t};  num_t = d*num_{t-1} + e^{k_t} v_t
        y_t   = (num_{t-1} + e^{k_t+u} v_t) / (den_{t-1} + e^{k_t+u})
  - Both den/num recurrences run on the DVE via 1024-wide
    tensor_tensor_scan (scans are DVE-only; ~2-3ns/elem on HW).
  - Sigmoid and division fused through logs in ONE activation table
    (natural_log_exp: Exp+Ln), so there are zero table reloads and no
    ACT ordering constraints:
        y = nm * exp(-(ln(dn) + ln(1 + e^{-r})))
          = sigmoid(r) * nm / dn
    ACT ops: ek=exp(k), em=exp(-r), lt=ln(1+em) (input bias),
    ln=ln(dn), rc=exp(-(ln+lt)); all free-ordered by the scheduler.
  - e^{k+u} and e^{k+u}*v never exp'd twice: ekb = ek*e^u and
    ekbv = ekv*e^u via DVE tensor_scalar with per-partition e^u.
  - Engine split (latency-chain aware): DVE scans+ekv+ekb+ekbv+dn+
    (ln+lt add)+y, GpSimd only nm (off the critical path), ACT all
    exp/ln + psum->sbuf copies; one shared 8-deep PSUM rotation for
    all matmul outputs.
  - Forward-direction y stays resident in SBUF (no HBM staging);
    output written bf16 (host upcasts).
"""

import numpy as np
import ml_dtypes

B, T, C = 8, 4096, 512
TT = 512           # time tile (psum width)
CB = 4             # channel blocks
PW = 2 * TT        # pair width for SBUF-side elementwise
NP = T // PW       # 4 pairs

NUM_SCAN_ENGINE = "vector"   # scans are DVE-only (TensorScalarPtr illegal on Pool)

_CACHE = {}


def _apply_tile_patches():
    """walrus in this container rejects instructions with >1 sync wait
    ("Too many sync wait commands"). Split excess waits onto same-engine
    nop carriers, and do the same for the TileContext tail drain."""
    import concourse.tile as tile_mod
    from concourse import mybir
    from concourse.vector_clock import ScopedClock

    if getattr(tile_mod, "_wait_split_patched", False):
        return
    MAXW = 1

    _orig_add = tile_mod.TileContext._add_instruction

    def _split_add(self, inst):
        si = inst.sync_info
        if si is not None and si.on_wait and len(si.on_wait) > MAXW:
            waits = list(si.on_wait)
            k = 0
            while len(waits) > MAXW:
                chunk, waits = waits[:MAXW], waits[MAXW:]
                carrier = mybir.InstNoOp(
                    name=f"{inst.name}_wsplit{k}",
                    engine=inst.engine,
                    bass_nofuse=True,
                    sync_info=mybir.SyncInfo(on_wait=chunk, on_update=[]),
                )
                k += 1
                _orig_add(self, carrier)
            inst.sync_info = mybir.SyncInfo(
                on_wait=waits, on_update=list(si.on_update)
            )
        return _orig_add(self, inst)

    def _drain_and_barrier(self, tick_clock, wait_clock):
        drain_inst = self.nc.sync.drain()
        wait_clock.add_sem_waits(
            drain_inst.ins, ScopedClock({None: tick_clock.global_clock})
        )
        si = drain_inst.ins.sync_info
        if si is not None and si.on_wait and len(si.on_wait) > MAXW:
            waits = list(si.on_wait)
            drain_inst.ins.sync_info = mybir.SyncInfo(
                on_wait=waits[:MAXW], on_update=list(si.on_update)
            )
            rest = waits[MAXW:]
            while rest:
                chunk, rest = rest[:MAXW], rest[MAXW:]
                n = self.nc.sync.nop(nofuse=True)
                n.ins.sync_info = mybir.SyncInfo(on_wait=chunk, on_update=[])

        self.nc.all_engine_barrier()
        assert self.sems is not None
        popped = self.nc._tile_sem_poison_stack.pop()
        assert popped is self._sem_poison
        self.nc.clear_and_free_semaphores(list(self.sems.allocated().values()))
        self.nc.all_engine_barrier()

    tile_mod.TileContext._add_instruction = _split_add
    tile_mod.TileContext._drain_and_barrier = _drain_and_barrier
    tile_mod._wait_split_patched = True


def _build_nc():
    import concourse.bass as bass
    import concourse.tile as tile
    from concourse import mybir
    from concourse.bass import _add_dep_helper

    _apply_tile_patches()

    f32 = mybir.dt.float32
    bf16 = mybir.dt.bfloat16
    Alu = mybir.AluOpType
    Act = mybir.ActivationFunctionType

    nc = bass.Bass()

    xT = nc.dram_tensor("xT", [C, T], bf16, kind="ExternalInput")
    wnames = ["w_rf", "w_kf", "w_vf", "w_rb", "w_kb", "w_vb"]
    wdram = {
        n: nc.dram_tensor(n, [128, 4 * C], bf16, kind="ExternalInput")
        for n in wnames
    }
    wout_d = nc.dram_tensor("wout", [128, 8 * C], bf16, kind="ExternalInput")
    u_f_d = nc.dram_tensor("u_f", [C, 1], f32, kind="ExternalInput")
    u_b_d = nc.dram_tensor("u_b", [C, 1], f32, kind="ExternalInput")
    eu_f_d = nc.dram_tensor("eu_f", [C, 1], f32, kind="ExternalInput")
    eu_b_d = nc.dram_tensor("eu_b", [C, 1], f32, kind="ExternalInput")
    dec_f_d = nc.dram_tensor("dec_f", [C, 1], f32, kind="ExternalInput")
    dec_b_d = nc.dram_tensor("dec_b", [C, 1], f32, kind="ExternalInput")
    out_d = nc.dram_tensor("y", [T, C], bf16, kind="ExternalOutput")

    # All activations use one table set (Exp+Ln), so no program-order
    # chaining is needed to avoid table reloads.
    def act(*args, chain=None, **kwargs):
        return nc.scalar.activation(*args, **kwargs)

    def act_copy(out, in_, chain=None):
        return nc.scalar.copy(out, in_)

    num_eng = nc.gpsimd if NUM_SCAN_ENGINE == "gpsimd" else nc.vector

    with tile.TileContext(nc) as tc:
        with (
            tc.tile_pool(name="wp", bufs=1) as wp,
            tc.tile_pool(name="cst", bufs=1) as cst,
            tc.tile_pool(name="ypf", bufs=1) as ypfp,
            tc.tile_pool(name="chain", bufs=2) as chainp,
            tc.tile_pool(name="xt", bufs=2) as xtp,
            tc.tile_pool(name="wk", bufs=1) as wkp,
            tc.tile_pool(name="ps", bufs=1, space="PSUM") as psp,
        ):
            # ---- resident weights & constants ----
            wout = wp.tile([128, 8 * C], bf16, name="wout")
            nc.sync.dma_start(wout[:], wout_d[:])
            wt = {}
            for n in wnames:
                wt[n] = wp.tile([128, 4 * C], bf16, tag=n, name=n)
                nc.sync.dma_start(wt[n][:], wdram[n][:])
            u_t, eu_t, dec_t = {}, {}, {}
            for cb in range(CB):
                sl = slice(cb * 128, (cb + 1) * 128)
                for d, ud, eud, dd in (("f", u_f_d, eu_f_d, dec_f_d),
                                       ("b", u_b_d, eu_b_d, dec_b_d)):
                    u_t[(d, cb)] = cst.tile([128, 1], f32, tag=f"u{d}{cb}",
                                            name=f"u{d}{cb}")
                    nc.sync.dma_start(u_t[(d, cb)][:], ud[sl, :])
                    eu_t[(d, cb)] = cst.tile([128, 1], f32, tag=f"e{d}{cb}",
                                             name=f"e{d}{cb}")
                    nc.sync.dma_start(eu_t[(d, cb)][:], eud[sl, :])
                    dec_t[(d, cb)] = cst.tile([128, 1], f32, tag=f"d{d}{cb}",
                                              name=f"d{d}{cb}")
                    nc.sync.dma_start(dec_t[(d, cb)][:], dd[sl, :])

            # forward-direction y, resident in SBUF across both phases
            ypf = {}
            for cb in range(CB):
                ypf[cb] = ypfp.tile([128, T], bf16, tag=f"ypf{cb}",
                                    name=f"ypf{cb}")

            def run_phase(d):
                fwd = d == "f"
                wr, wk, wv = wt["w_r" + d], wt["w_k" + d], wt["w_v" + d]
                pairs = list(range(NP)) if fwd else list(reversed(range(NP)))
                chains = {}

                def chain_buf(cb, kind):
                    key = (cb, kind)
                    t = chainp.tile([128, PW + 1], bf16,
                                    tag=f"ch_{kind}{cb}",
                                    name=f"ch_{kind}{cb}")
                    prev = chains.get(key)
                    chains[key] = t
                    eng = num_eng if kind == "num" else nc.vector
                    if fwd:
                        if prev is None:
                            eng.memset(t[:, 0:1], 0.0)
                        else:
                            eng.tensor_copy(t[:, 0:1], prev[:, PW: PW + 1])
                    else:
                        if prev is None:
                            eng.memset(t[:, PW: PW + 1], 0.0)
                        else:
                            eng.tensor_copy(t[:, PW: PW + 1], prev[:, 0:1])
                    return t

                stash = {}     # (pr, cb) -> (dn, sgm)
                ypb_tiles = {}  # (pr, cb) -> y tile (bwd only)

                def part_a(pr):
                    p0 = pr * PW
                    xts = {}
                    for half, tt in enumerate((2 * pr, 2 * pr + 1)):
                        t0 = tt * TT
                        for kb in range(4):
                            xt = xtp.tile([128, TT], bf16,
                                          tag=f"xt{kb}h{half}",
                                          bufs=2, name=f"xt{kb}h{half}")
                            nc.sync.dma_start(
                                xt[:],
                                xT[kb * 128:(kb + 1) * 128, t0: t0 + TT])
                            xts[(half, kb)] = xt
                    for cb in range(CB):
                        pss = {}
                        for cls, w in (("k", wk), ("v", wv), ("r", wr)):
                            for half in range(2):
                                pss[(cls, half)] = psp.tile(
                                    [128, TT], f32, tag="ps", bufs=8,
                                    name=f"ps{cls}")
                            for kb in range(4):
                                wsl = w[:, kb * C + cb * 128:
                                        kb * C + cb * 128 + 128]
                                for half in range(2):
                                    nc.tensor.matmul(
                                        pss[(cls, half)][:], wsl,
                                        xts[(half, kb)][:],
                                        start=(kb == 0), stop=(kb == 3))
                        # 1024-wide work tiles, written per half
                        ek = wkp.tile([128, PW], bf16, tag="ek", bufs=2,
                                      name="ek")
                        ekb = wkp.tile([128, PW], bf16, tag="ekb", bufs=2,
                                       name="ekb")
                        em = wkp.tile([128, PW], bf16, tag="em", bufs=2,
                                      name="em")
                        ekv = wkp.tile([128, PW], bf16, tag="ekv", bufs=2,
                                       name="ekv")
                        vc = wkp.tile([128, PW], bf16, tag="vc", bufs=2,
                                      name="vc")
                        for half in range(2):
                            hs = slice(half * TT, (half + 1) * TT)
                            act(ek[:, hs], pss[("k", half)][:], Act.Exp)
                            act(em[:, hs], pss[("r", half)][:], Act.Exp,
                                bias=0.0, scale=-1.0)
                            act_copy(vc[:, hs], pss[("v", half)][:])
                        nc.vector.tensor_mul(ekv[:], ek[:], vc[:])
                        lt = wkp.tile([128, PW], f32, tag="lt", bufs=4,
                                      name="lt")
                        act(lt[:], em[:], Act.Ln, bias=1.0)
                        nc.vector.tensor_scalar_mul(
                            ekb[:], ek[:], eu_t[(d, cb)][:, 0:1])
                        # scans on DVE (full pair width)
                        decbc = dec_t[(d, cb)][:, 0:1].broadcast_to([128, PW])
                        denb = chain_buf(cb, "den")
                        numb = chain_buf(cb, "num")
                        if fwd:
                            nc.vector.tensor_tensor_scan(
                                denb[:, 1: 1 + PW], decbc, ek[:],
                                denb[:, 0:1], Alu.mult, Alu.add)
                            num_eng.tensor_tensor_scan(
                                numb[:, 1: 1 + PW], decbc, ekv[:],
                                numb[:, 0:1], Alu.mult, Alu.add)
                            den_prev = denb[:, 0:PW]
                            num_prev = numb[:, 0:PW]
                        else:
                            nc.vector.tensor_tensor_scan(
                                denb[:, 0:PW][:, ::-1], decbc,
                                ek[:][:, ::-1], denb[:, PW: PW + 1],
                                Alu.mult, Alu.add)
                            num_eng.tensor_tensor_scan(
                                numb[:, 0:PW][:, ::-1], decbc,
                                ekv[:][:, ::-1], numb[:, PW: PW + 1],
                                Alu.mult, Alu.add)
                            den_prev = denb[:, 1: 1 + PW]
                            num_prev = numb[:, 1: 1 + PW]
                        dn = wkp.tile([128, PW], bf16, tag="dn", bufs=4,
                                      name="dn")
                        ekbv = wkp.tile([128, PW], bf16, tag="ekbv", bufs=2,
                                        name="ekbv")
                        nm = wkp.tile([128, PW], bf16, tag="nm", bufs=4,
                                      name="nm")
                        nc.vector.tensor_add(dn[:], ekb[:], den_prev)
                        nc.vector.tensor_scalar_mul(
                            ekbv[:], ekv[:], eu_t[(d, cb)][:, 0:1])
                        nc.gpsimd.tensor_add(nm[:], ekbv[:], num_prev)
                        stash[(pr, cb)] = (dn, nm, lt)

                def part_b(pr):
                    p0 = pr * PW
                    for cb in range(CB):
                        dn, nm, lt = stash.pop((pr, cb))
                        ln = wkp.tile([128, PW], f32, tag="ln", bufs=2,
                                      name="ln")
                        act(ln[:], dn[:], Act.Ln)
                        sl_t = wkp.tile([128, PW], f32, tag="sl", bufs=2,
                                        name="sl")
                        nc.vector.tensor_add(sl_t[:], ln[:], lt[:])
                        rc = wkp.tile([128, PW], bf16, tag="rc", bufs=2,
                                      name="rc")
                        act(rc[:], sl_t[:], Act.Exp, scale=-1.0)
                        if fwd:
                            nc.vector.tensor_mul(
                                ypf[cb][:, p0: p0 + PW], nm[:], rc[:])
                        else:
                            yb = wkp.tile([128, PW], bf16, tag=f"ypb{cb}",
                                          bufs=2, name=f"ypb{cb}")
                            nc.vector.tensor_mul(yb[:], nm[:], rc[:])
                            ypb_tiles[(pr, cb)] = yb

                def part_c(pr):
                    p0 = pr * PW
                    for m in range(PW // 128):
                        t0 = p0 + m * 128
                        pso = psp.tile([128, C], f32, tag="ps", bufs=8,
                                       name="pso")
                        for cb in range(CB):
                            nc.tensor.matmul(
                                pso[:],
                                ypf[cb][:, t0: t0 + 128],
                                wout[:, cb * C: (cb + 1) * C],
                                start=(cb == 0), stop=False)
                        for cb in range(CB):
                            nc.tensor.matmul(
                                pso[:],
                                ypb_tiles[(pr, cb)][:, m * 128:
                                                    (m + 1) * 128],
                                wout[:, (4 + cb) * C: (5 + cb) * C],
                                start=False, stop=(cb == 3))
                        osb = wkp.tile([128, C], bf16, tag="osb",
                                       bufs=2, name="osb")
                        act_copy(osb[:], pso[:])
                        nc.sync.dma_start(out_d[t0: t0 + 128, :], osb[:])

                # group pairs in twos: A A B B (C C) to batch the
                # Reciprocal table epoch across 2 pairs
                for pr in pairs:
                    part_a(pr)
                    part_b(pr)
                    if not fwd:
                        part_c(pr)
                        for key in list(ypb_tiles):
                            if key[0] == pr:
                                del ypb_tiles[key]

            run_phase("f")
            run_phase("b")

    return nc


def _host_prep(x, W_rkv, W_out, time_decay, time_first, time_decay_rev,
               time_first_rev):
    bf16 = ml_dtypes.bfloat16
    f32 = np.float32

    Wr = W_rkv.reshape(C, 2, 3, C)
    pieces = {
        "w_rf": Wr[:, 0, 0], "w_kf": Wr[:, 0, 1], "w_vf": Wr[:, 0, 2],
        "w_rb": Wr[:, 1, 0], "w_kb": Wr[:, 1, 1], "w_vb": Wr[:, 1, 2],
    }
    wmaps = {}
    for n, p in pieces.items():
        wmaps[n] = np.ascontiguousarray(
            p.reshape(4, 128, C).transpose(1, 0, 2).reshape(128, 4 * C)
        ).astype(bf16)

    Wo = W_out.reshape(8, 128, C).transpose(1, 0, 2).reshape(128, 8 * C)
    wout = np.ascontiguousarray(Wo).astype(bf16)

    u_f = np.ascontiguousarray(time_first.reshape(C, 1)).astype(f32)
    u_b = np.ascontiguousarray(time_first_rev.reshape(C, 1)).astype(f32)
    eu_f = np.exp(time_first.astype(np.float64)).reshape(C, 1).astype(f32)
    eu_b = np.exp(time_first_rev.astype(np.float64)).reshape(C, 1).astype(f32)
    dec_f = np.exp(-np.exp(time_decay.astype(np.float64))).reshape(C, 1).astype(f32)
    dec_b = np.exp(-np.exp(time_decay_rev.astype(np.float64))).reshape(C, 1).astype(f32)

    shared = dict(wout=wout, u_f=u_f, u_b=u_b, eu_f=eu_f, eu_b=eu_b,
                  dec_f=dec_f, dec_b=dec_b, **wmaps)
    in_maps = []
    for b in range(B):
        m = dict(shared)
        m["xT"] = np.ascontiguousarray(x[b].T).astype(bf16)
        in_maps.append(m)
    return in_maps


def kernel(x, W_rkv, W_out, time_decay, time_first, time_decay_rev,
           time_first_rev, _trace=False):
    from concourse.bass_utils import run_bass_kernel_spmd

    x = np.asarray(x, dtype=np.float32)
    W_rkv = np.asarray(W_rkv, dtype=np.float32)
    W_out = np.asarray(W_out, dtype=np.float32)
    time_decay = np.asarray(time_decay, dtype=np.float32)
    time_first = np.asarray(time_first, dtype=np.float32)
    time_decay_rev = np.asarray(time_decay_rev, dtype=np.float32)
    time_first_rev = np.asarray(time_first_rev, dtype=np.float32)

    if "nc" not in _CACHE:
        _CACHE["nc"] = _build_nc()
    nc = _CACHE["nc"]

    in_maps = _host_prep(x, W_rkv, W_out, time_decay, time_first,
                         time_decay_rev, time_first_rev)
    res = run_bass_kernel_spmd(
        nc, in_maps, core_ids=list(range(B)), trace=_trace
    )
    _CACHE["last_result"] = res
    out = np.stack([res.results[b]["y"].astype(np.float32) for b in range(B)])
    return out


# revision 36
# speedup vs baseline: 1.1898x; 1.1898x over previous
"""BiRWKV layer kernel for 8 Trainium2 NeuronCores.

Strategy (data-parallel over B=8, one batch element per core):
  - (channel, time) layout on chip: channels on the 128 SBUF partitions
    (C=512 -> 4 blocks), time on the free dim.
  - r/k/v projections for both directions are bf16 matmuls
    (lhsT = W block, rhs = x^T block) accumulated over 4 input-channel
    blocks into PSUM (fp32).
  - WKV runs UNSTABILIZED (mathematically equal to the reference's
    log-sum-exp form; values stay in range since |w|*T <= ~28, k~N(0,1)):
        den_t = d*den_{t-1} + e^{k_t};  num_t = d*num_{t-1} + e^{k_t} v_t
        y_t   = (num_{t-1} + e^{k_t+u} v_t) / (den_{t-1} + e^{k_t+u})
  - Both den/num recurrences run on the DVE via 1024-wide
    tensor_tensor_scan (scans are DVE-only; ~2-3ns/elem on HW).
  - Sigmoid and division fused through logs in ONE activation table
    (natural_log_exp: Exp+Ln), so there are zero table reloads and no
    ACT ordering constraints:
        y = nm * exp(-(ln(dn) + ln(1 + e^{-r})))
          = sigmoid(r) * nm / dn
    ACT ops: ek=exp(k), em=exp(-r), lt=ln(1+em) (input bias),
    ln=ln(dn), rc=exp(-(ln+lt)); all free-ordered by the scheduler.
  - e^{k+u} and e^{k+u}*v never exp'd twice: ekb = ek*e^u and
    ekbv = ekv*e^u via DVE tensor_scalar with per-partition e^u.
  - Engine split (latency-chain aware): DVE scans+ekv+ekb+ekbv+dn+
    (ln+lt add)+y, GpSimd only nm (off the critical path), ACT all
    exp/ln + psum->sbuf copies; one shared 8-deep PSUM rotation for
    all matmul outputs.
  - Forward-direction y stays resident in SBUF (no HBM staging);
    output written bf16 (host upcasts).
"""

import numpy as np
import ml_dtypes

B, T, C = 8, 4096, 512
TT = 512           # time tile (psum width)
CB = 4             # channel blocks
PW = 2 * TT        # pair width for SBUF-side elementwise
NP = T // PW       # 4 pairs

NUM_SCAN_ENGINE = "vector"   # scans are DVE-only (TensorScalarPtr illegal on Pool)

_CACHE = {}


def _apply_tile_patches():
    """walrus in this container rejects instructions with >1 sync wait
    ("Too many sync wait commands"). Split excess waits onto same-engine
    nop carriers, and do the same for the TileContext tail drain."""
    import concourse.tile as tile_mod
    from concourse import mybir
    from concourse.vector_clock import ScopedClock

    if getattr(tile_mod, "_wait_split_patched", False):
        return
    MAXW = 1

    _orig_add = tile_mod.TileContext._add_instruction

    def _split_add(self, inst):
        si = inst.sync_info
        if si is not None and si.on_wait and len(si.on_wait) > MAXW:
            waits = list(si.on_wait)
            k = 0
            while len(waits) > MAXW:
                chunk, waits = waits[:MAXW], waits[MAXW:]
                carrier = mybir.InstNoOp(
                    name=f"{inst.name}_wsplit{k}",
                    engine=inst.engine,
                    bass_nofuse=True,
                    sync_info=mybir.SyncInfo(on_wait=chunk, on_update=[]),
                )
                k += 1
                _orig_add(self, carrier)
            inst.sync_info = mybir.SyncInfo(
                on_wait=waits, on_update=list(si.on_update)
            )
        return _orig_add(self, inst)

    def _drain_and_barrier(self, tick_clock, wait_clock):
        drain_inst = self.nc.sync.drain()
        wait_clock.add_sem_waits(
            drain_inst.ins, ScopedClock({None: tick_clock.global_clock})
        )
        si = drain_inst.ins.sync_info
        if si is not None and si.on_wait and len(si.on_wait) > MAXW:
            waits = list(si.on_wait)
            drain_inst.ins.sync_info = mybir.SyncInfo(
                on_wait=waits[:MAXW], on_update=list(si.on_update)
            )
            rest = waits[MAXW:]
            while rest:
                chunk, rest = rest[:MAXW], rest[MAXW:]
                n = self.nc.sync.nop(nofuse=True)
                n.ins.sync_info = mybir.SyncInfo(on_wait=chunk, on_update=[])

        self.nc.all_engine_barrier()
        assert self.sems is not None
        popped = self.nc._tile_sem_poison_stack.pop()
        assert popped is self._sem_poison
        self.nc.clear_and_free_semaphores(list(self.sems.allocated().values()))
        self.nc.all_engine_barrier()

    tile_mod.TileContext._add_instruction = _split_add
    tile_mod.TileContext._drain_and_barrier = _drain_and_barrier
    tile_mod._wait_split_patched = True


def _build_nc():
    import concourse.bass as bass
    import concourse.tile as tile
    from concourse import mybir
    from concourse.bass import _add_dep_helper

    _apply_tile_patches()

    f32 = mybir.dt.float32
    bf16 = mybir.dt.bfloat16
    Alu = mybir.AluOpType
    Act = mybir.ActivationFunctionType

    nc = bass.Bass()

    xT = nc.dram_tensor("xT", [C, T], bf16, kind="ExternalInput")
    wnames = ["w_rf", "w_kf", "w_vf", "w_rb", "w_kb", "w_vb"]
    wdram = {
        n: nc.dram_tensor(n, [128, 4 * C], bf16, kind="ExternalInput")
        for n in wnames
    }
    wout_d = nc.dram_tensor("wout", [128, 8 * C], bf16, kind="ExternalInput")
    u_f_d = nc.dram_tensor("u_f", [C, 1], f32, kind="ExternalInput")
    u_b_d = nc.dram_tensor("u_b", [C, 1], f32, kind="ExternalInput")
    eu_f_d = nc.dram_tensor("eu_f", [C, 1], f32, kind="ExternalInput")
    eu_b_d = nc.dram_tensor("eu_b", [C, 1], f32, kind="ExternalInput")
    dec_f_d = nc.dram_tensor("dec_f", [C, 1], f32, kind="ExternalInput")
    dec_b_d = nc.dram_tensor("dec_b", [C, 1], f32, kind="ExternalInput")
    out_d = nc.dram_tensor("y", [T, C], bf16, kind="ExternalOutput")

    # All activations use one table set (Exp+Ln), so no program-order
    # chaining is needed to avoid table reloads.
    def act(*args, chain=None, **kwargs):
        return nc.scalar.activation(*args, **kwargs)

    def act_copy(out, in_, chain=None):
        return nc.scalar.copy(out, in_)

    num_eng = nc.gpsimd if NUM_SCAN_ENGINE == "gpsimd" else nc.vector

    with tile.TileContext(nc) as tc:
        with (
            tc.tile_pool(name="wp", bufs=1) as wp,
            tc.tile_pool(name="cst", bufs=1) as cst,
            tc.tile_pool(name="ypf", bufs=1) as ypfp,
            tc.tile_pool(name="chain", bufs=2) as chainp,
            tc.tile_pool(name="xt", bufs=2) as xtp,
            tc.tile_pool(name="wk", bufs=1) as wkp,
            tc.tile_pool(name="ps", bufs=1, space="PSUM") as psp,
        ):
            # ---- resident weights & constants ----
            wout = wp.tile([128, 8 * C], bf16, name="wout")
            nc.sync.dma_start(wout[:], wout_d[:])
            wt = {}
            for n in wnames:
                wt[n] = wp.tile([128, 4 * C], bf16, tag=n, name=n)
                nc.sync.dma_start(wt[n][:], wdram[n][:])
            u_t, eu_t, dec_t = {}, {}, {}
            for cb in range(CB):
                sl = slice(cb * 128, (cb + 1) * 128)
                for d, ud, eud, dd in (("f", u_f_d, eu_f_d, dec_f_d),
                                       ("b", u_b_d, eu_b_d, dec_b_d)):
                    u_t[(d, cb)] = cst.tile([128, 1], f32, tag=f"u{d}{cb}",
                                            name=f"u{d}{cb}")
                    nc.sync.dma_start(u_t[(d, cb)][:], ud[sl, :])
                    eu_t[(d, cb)] = cst.tile([128, 1], f32, tag=f"e{d}{cb}",
                                             name=f"e{d}{cb}")
                    nc.sync.dma_start(eu_t[(d, cb)][:], eud[sl, :])
                    dec_t[(d, cb)] = cst.tile([128, 1], f32, tag=f"d{d}{cb}",
                                              name=f"d{d}{cb}")
                    nc.sync.dma_start(dec_t[(d, cb)][:], dd[sl, :])

            # forward-direction y, resident in SBUF across both phases
            ypf = {}
            for cb in range(CB):
                ypf[cb] = ypfp.tile([128, T], bf16, tag=f"ypf{cb}",
                                    name=f"ypf{cb}")

            def run_phase(d):
                fwd = d == "f"
                wr, wk, wv = wt["w_r" + d], wt["w_k" + d], wt["w_v" + d]
                pairs = list(range(NP)) if fwd else list(reversed(range(NP)))
                chains = {}

                def chain_buf(cb, kind):
                    key = (cb, kind)
                    t = chainp.tile([128, PW + 1], bf16,
                                    tag=f"ch_{kind}{cb}",
                                    name=f"ch_{kind}{cb}")
                    prev = chains.get(key)
                    chains[key] = t
                    eng = num_eng if kind == "num" else nc.vector
                    if fwd:
                        if prev is None:
                            eng.memset(t[:, 0:1], 0.0)
                        else:
                            eng.tensor_copy(t[:, 0:1], prev[:, PW: PW + 1])
                    else:
                        if prev is None:
                            eng.memset(t[:, PW: PW + 1], 0.0)
                        else:
                            eng.tensor_copy(t[:, PW: PW + 1], prev[:, 0:1])
                    return t

                stash = {}     # (pr, cb) -> (dn, sgm)
                ypb_tiles = {}  # (pr, cb) -> y tile (bwd only)

                def part_a(pr):
                    p0 = pr * PW
                    xts = {}
                    for half, tt in enumerate((2 * pr, 2 * pr + 1)):
                        t0 = tt * TT
                        for kb in range(4):
                            xt = xtp.tile([128, TT], bf16,
                                          tag=f"xt{kb}h{half}",
                                          bufs=2, name=f"xt{kb}h{half}")
                            nc.sync.dma_start(
                                xt[:],
                                xT[kb * 128:(kb + 1) * 128, t0: t0 + TT])
                            xts[(half, kb)] = xt
                    for cb in range(CB):
                        pss = {}
                        for cls, w in (("k", wk), ("v", wv), ("r", wr)):
                            for half in range(2):
                                pss[(cls, half)] = psp.tile(
                                    [128, TT], f32, tag="ps", bufs=8,
                                    name=f"ps{cls}")
                            for kb in range(4):
                                wsl = w[:, kb * C + cb * 128:
                                        kb * C + cb * 128 + 128]
                                for half in range(2):
                                    nc.tensor.matmul(
                                        pss[(cls, half)][:], wsl,
                                        xts[(half, kb)][:],
                                        start=(kb == 0), stop=(kb == 3))
                        # 1024-wide work tiles, written per half
                        ek = wkp.tile([128, PW], bf16, tag="ek", bufs=2,
                                      name="ek")
                        ekb = wkp.tile([128, PW], bf16, tag="ekb", bufs=2,
                                       name="ekb")
                        em = wkp.tile([128, PW], bf16, tag="em", bufs=2,
                                      name="em")
                        ekv = wkp.tile([128, PW], bf16, tag="ekv", bufs=2,
                                       name="ekv")
                        for half in range(2):
                            hs = slice(half * TT, (half + 1) * TT)
                            act(ek[:, hs], pss[("k", half)][:], Act.Exp)
                            act(em[:, hs], pss[("r", half)][:], Act.Exp,
                                bias=0.0, scale=-1.0)
                            nc.vector.tensor_mul(ekv[:, hs], ek[:, hs],
                                                 pss[("v", half)][:])
                        lt = wkp.tile([128, PW], f32, tag="lt", bufs=4,
                                      name="lt")
                        act(lt[:], em[:], Act.Ln, bias=1.0)
                        nc.vector.tensor_scalar_mul(
                            ekb[:], ek[:], eu_t[(d, cb)][:, 0:1])
                        # scans on DVE (full pair width)
                        decbc = dec_t[(d, cb)][:, 0:1].broadcast_to([128, PW])
                        denb = chain_buf(cb, "den")
                        numb = chain_buf(cb, "num")
                        if fwd:
                            nc.vector.tensor_tensor_scan(
                                denb[:, 1: 1 + PW], decbc, ek[:],
                                denb[:, 0:1], Alu.mult, Alu.add)
                            num_eng.tensor_tensor_scan(
                                numb[:, 1: 1 + PW], decbc, ekv[:],
                                numb[:, 0:1], Alu.mult, Alu.add)
                            den_prev = denb[:, 0:PW]
                            num_prev = numb[:, 0:PW]
                        else:
                            nc.vector.tensor_tensor_scan(
                                denb[:, 0:PW][:, ::-1], decbc,
                                ek[:][:, ::-1], denb[:, PW: PW + 1],
                                Alu.mult, Alu.add)
                            num_eng.tensor_tensor_scan(
                                numb[:, 0:PW][:, ::-1], decbc,
                                ekv[:][:, ::-1], numb[:, PW: PW + 1],
                                Alu.mult, Alu.add)
                            den_prev = denb[:, 1: 1 + PW]
                            num_prev = numb[:, 1: 1 + PW]
                        dn = wkp.tile([128, PW], bf16, tag="dn", bufs=4,
                                      name="dn")
                        ekbv = wkp.tile([128, PW], bf16, tag="ekbv", bufs=2,
                                        name="ekbv")
                        nm = wkp.tile([128, PW], bf16, tag="nm", bufs=4,
                                      name="nm")
                        nc.vector.tensor_add(dn[:], ekb[:], den_prev)
                        nc.vector.tensor_scalar_mul(
                            ekbv[:], ekv[:], eu_t[(d, cb)][:, 0:1])
                        nc.gpsimd.tensor_add(nm[:], ekbv[:], num_prev)
                        stash[(pr, cb)] = (dn, nm, lt)

                def part_b(pr):
                    p0 = pr * PW
                    for cb in range(CB):
                        dn, nm, lt = stash.pop((pr, cb))
                        ln = wkp.tile([128, PW], f32, tag="ln", bufs=2,
                                      name="ln")
                        act(ln[:], dn[:], Act.Ln)
                        sl_t = wkp.tile([128, PW], f32, tag="sl", bufs=2,
                                        name="sl")
                        nc.vector.tensor_add(sl_t[:], ln[:], lt[:])
                        rc = wkp.tile([128, PW], bf16, tag="rc", bufs=2,
                                      name="rc")
                        act(rc[:], sl_t[:], Act.Exp, scale=-1.0)
                        if fwd:
                            nc.gpsimd.tensor_mul(
                                ypf[cb][:, p0: p0 + PW], nm[:], rc[:])
                        else:
                            yb = wkp.tile([128, PW], bf16, tag=f"ypb{cb}",
                                          bufs=2, name=f"ypb{cb}")
                            nc.vector.tensor_mul(yb[:], nm[:], rc[:])
                            ypb_tiles[(pr, cb)] = yb

                def part_c(pr):
                    p0 = pr * PW
                    for m in range(PW // 128):
                        t0 = p0 + m * 128
                        pso = psp.tile([128, C], f32, tag="ps", bufs=8,
                                       name="pso")
                        for cb in range(CB):
                            nc.tensor.matmul(
                                pso[:],
                                ypf[cb][:, t0: t0 + 128],
                                wout[:, cb * C: (cb + 1) * C],
                                start=(cb == 0), stop=False)
                        for cb in range(CB):
                            nc.tensor.matmul(
                                pso[:],
                                ypb_tiles[(pr, cb)][:, m * 128:
                                                    (m + 1) * 128],
                                wout[:, (4 + cb) * C: (5 + cb) * C],
                                start=False, stop=(cb == 3))
                        osb = wkp.tile([128, C], bf16, tag="osb",
                                       bufs=2, name="osb")
                        act_copy(osb[:], pso[:])
                        nc.sync.dma_start(out_d[t0: t0 + 128, :], osb[:])

                # group pairs in twos: A A B B (C C) to batch the
                # Reciprocal table epoch across 2 pairs
                for pr in pairs:
                    part_a(pr)
                    part_b(pr)
                    if not fwd:
                        part_c(pr)
                        for key in list(ypb_tiles):
                            if key[0] == pr:
                                del ypb_tiles[key]

            run_phase("f")
            run_phase("b")

    return nc


def _host_prep(x, W_rkv, W_out, time_decay, time_first, time_decay_rev,
               time_first_rev):
    bf16 = ml_dtypes.bfloat16
    f32 = np.float32

    Wr = W_rkv.reshape(C, 2, 3, C)
    pieces = {
        "w_rf": Wr[:, 0, 0], "w_kf": Wr[:, 0, 1], "w_vf": Wr[:, 0, 2],
        "w_rb": Wr[:, 1, 0], "w_kb": Wr[:, 1, 1], "w_vb": Wr[:, 1, 2],
    }
    wmaps = {}
    for n, p in pieces.items():
        wmaps[n] = np.ascontiguousarray(
            p.reshape(4, 128, C).transpose(1, 0, 2).reshape(128, 4 * C)
        ).astype(bf16)

    Wo = W_out.reshape(8, 128, C).transpose(1, 0, 2).reshape(128, 8 * C)
    wout = np.ascontiguousarray(Wo).astype(bf16)

    u_f = np.ascontiguousarray(time_first.reshape(C, 1)).astype(f32)
    u_b = np.ascontiguousarray(time_first_rev.reshape(C, 1)).astype(f32)
    eu_f = np.exp(time_first.astype(np.float64)).reshape(C, 1).astype(f32)
    eu_b = np.exp(time_first_rev.astype(np.float64)).reshape(C, 1).astype(f32)
    dec_f = np.exp(-np.exp(time_decay.astype(np.float64))).reshape(C, 1).astype(f32)
    dec_b = np.exp(-np.exp(time_decay_rev.astype(np.float64))).reshape(C, 1).astype(f32)

    shared = dict(wout=wout, u_f=u_f, u_b=u_b, eu_f=eu_f, eu_b=eu_b,
                  dec_f=dec_f, dec_b=dec_b, **wmaps)
    in_maps = []
    for b in range(B):
        m = dict(shared)
        m["xT"] = np.ascontiguousarray(x[b].T).astype(bf16)
        in_maps.append(m)
    return in_maps


def kernel(x, W_rkv, W_out, time_decay, time_first, time_decay_rev,
           time_first_rev, _trace=False):
    from concourse.bass_utils import run_bass_kernel_spmd

    x = np.asarray(x, dtype=np.float32)
    W_rkv = np.asarray(W_rkv, dtype=np.float32)
    W_out = np.asarray(W_out, dtype=np.float32)
    time_decay = np.asarray(time_decay, dtype=np.float32)
    time_first = np.asarray(time_first, dtype=np.float32)
    time_decay_rev = np.asarray(time_decay_rev, dtype=np.float32)
    time_first_rev = np.asarray(time_first_rev, dtype=np.float32)

    if "nc" not in _CACHE:
        _CACHE["nc"] = _build_nc()
    nc = _CACHE["nc"]

    in_maps = _host_prep(x, W_rkv, W_out, time_decay, time_first,
                         time_decay_rev, time_first_rev)
    res = run_bass_kernel_spmd(
        nc, in_maps, core_ids=list(range(B)), trace=_trace
    )
    _CACHE["last_result"] = res
    out = np.stack([res.results[b]["y"].astype(np.float32) for b in range(B)])
    return out


# revision 38
# speedup vs baseline: 1.2340x; 1.0372x over previous
"""BiRWKV layer kernel for 8 Trainium2 NeuronCores.

Strategy (data-parallel over B=8, one batch element per core):
  - (channel, time) layout on chip: channels on the 128 SBUF partitions
    (C=512 -> 4 blocks), time on the free dim.
  - r/k/v projections for both directions are bf16 matmuls
    (lhsT = W block, rhs = x^T block) accumulated over 4 input-channel
    blocks into PSUM (fp32).
  - WKV runs UNSTABILIZED (mathematically equal to the reference's
    log-sum-exp form; values stay in range since |w|*T <= ~28, k~N(0,1)):
        den_t = d*den_{t-1} + e^{k_t};  num_t = d*num_{t-1} + e^{k_t} v_t
        y_t   = (num_{t-1} + e^{k_t+u} v_t) / (den_{t-1} + e^{k_t+u})
  - Both den/num recurrences run on the DVE via 1024-wide
    tensor_tensor_scan (scans are DVE-only; ~2-3ns/elem on HW).
  - Sigmoid and division fused through logs in ONE activation table
    (natural_log_exp: Exp+Ln), so there are zero table reloads and no
    ACT ordering constraints:
        y = nm * exp(-(ln(dn) + ln(1 + e^{-r})))
          = sigmoid(r) * nm / dn
    ACT ops: ek=exp(k), em=exp(-r), lt=ln(1+em) (input bias),
    ln=ln(dn), rc=exp(-(ln+lt)); all free-ordered by the scheduler.
  - e^{k+u} and e^{k+u}*v never exp'd twice: ekb = ek*e^u and
    ekbv = ekv*e^u via DVE tensor_scalar with per-partition e^u.
  - Engine split (latency-chain aware): DVE scans+ekv+ekb+ekbv+dn+
    (ln+lt add)+y, GpSimd only nm (off the critical path), ACT all
    exp/ln + psum->sbuf copies; one shared 8-deep PSUM rotation for
    all matmul outputs.
  - Forward-direction y stays resident in SBUF (no HBM staging);
    output written bf16 (host upcasts).
"""

import numpy as np
import ml_dtypes

B, T, C = 8, 4096, 512
TT = 512           # time tile (psum width)
CB = 4             # channel blocks
PW = 2 * TT        # pair width for SBUF-side elementwise
NP = T // PW       # 4 pairs

NUM_SCAN_ENGINE = "vector"   # scans are DVE-only (TensorScalarPtr illegal on Pool)

_CACHE = {}


def _apply_tile_patches():
    """walrus in this container rejects instructions with >1 sync wait
    ("Too many sync wait commands"). Split excess waits onto same-engine
    nop carriers, and do the same for the TileContext tail drain."""
    import concourse.tile as tile_mod
    from concourse import mybir
    from concourse.vector_clock import ScopedClock

    if getattr(tile_mod, "_wait_split_patched", False):
        return
    MAXW = 1

    _orig_add = tile_mod.TileContext._add_instruction

    def _split_add(self, inst):
        si = inst.sync_info
        if si is not None and si.on_wait and len(si.on_wait) > MAXW:
            waits = list(si.on_wait)
            k = 0
            while len(waits) > MAXW:
                chunk, waits = waits[:MAXW], waits[MAXW:]
                carrier = mybir.InstNoOp(
                    name=f"{inst.name}_wsplit{k}",
                    engine=inst.engine,
                    bass_nofuse=True,
                    sync_info=mybir.SyncInfo(on_wait=chunk, on_update=[]),
                )
                k += 1
                _orig_add(self, carrier)
            inst.sync_info = mybir.SyncInfo(
                on_wait=waits, on_update=list(si.on_update)
            )
        return _orig_add(self, inst)

    def _drain_and_barrier(self, tick_clock, wait_clock):
        drain_inst = self.nc.sync.drain()
        wait_clock.add_sem_waits(
            drain_inst.ins, ScopedClock({None: tick_clock.global_clock})
        )
        si = drain_inst.ins.sync_info
        if si is not None and si.on_wait and len(si.on_wait) > MAXW:
            waits = list(si.on_wait)
            drain_inst.ins.sync_info = mybir.SyncInfo(
                on_wait=waits[:MAXW], on_update=list(si.on_update)
            )
            rest = waits[MAXW:]
            while rest:
                chunk, rest = rest[:MAXW], rest[MAXW:]
                n = self.nc.sync.nop(nofuse=True)
                n.ins.sync_info = mybir.SyncInfo(on_wait=chunk, on_update=[])

        self.nc.all_engine_barrier()
        assert self.sems is not None
        popped = self.nc._tile_sem_poison_stack.pop()
        assert popped is self._sem_poison
        self.nc.clear_and_free_semaphores(list(self.sems.allocated().values()))
        self.nc.all_engine_barrier()

    tile_mod.TileContext._add_instruction = _split_add
    tile_mod.TileContext._drain_and_barrier = _drain_and_barrier
    tile_mod._wait_split_patched = True


def _build_nc():
    import concourse.bass as bass
    import concourse.tile as tile
    from concourse import mybir
    from concourse.bass import _add_dep_helper

    _apply_tile_patches()

    f32 = mybir.dt.float32
    bf16 = mybir.dt.bfloat16
    Alu = mybir.AluOpType
    Act = mybir.ActivationFunctionType

    nc = bass.Bass()

    xT = nc.dram_tensor("xT", [C, T], bf16, kind="ExternalInput")
    wnames = ["w_rf", "w_kf", "w_vf", "w_rb", "w_kb", "w_vb"]
    wdram = {
        n: nc.dram_tensor(n, [128, 4 * C], bf16, kind="ExternalInput")
        for n in wnames
    }
    wout_d = nc.dram_tensor("wout", [128, 8 * C], bf16, kind="ExternalInput")
    u_f_d = nc.dram_tensor("u_f", [C, 1], f32, kind="ExternalInput")
    u_b_d = nc.dram_tensor("u_b", [C, 1], f32, kind="ExternalInput")
    eu_f_d = nc.dram_tensor("eu_f", [C, 1], f32, kind="ExternalInput")
    eu_b_d = nc.dram_tensor("eu_b", [C, 1], f32, kind="ExternalInput")
    dec_f_d = nc.dram_tensor("dec_f", [C, 1], f32, kind="ExternalInput")
    dec_b_d = nc.dram_tensor("dec_b", [C, 1], f32, kind="ExternalInput")
    out_d = nc.dram_tensor("y", [T, C], bf16, kind="ExternalOutput")

    # All activations use one table set (Exp+Ln), so no program-order
    # chaining is needed to avoid table reloads.
    def act(*args, chain=None, **kwargs):
        return nc.scalar.activation(*args, **kwargs)

    def act_copy(out, in_, chain=None):
        return nc.scalar.copy(out, in_)

    num_eng = nc.gpsimd if NUM_SCAN_ENGINE == "gpsimd" else nc.vector

    with tile.TileContext(nc) as tc:
        with (
            tc.tile_pool(name="wp", bufs=1) as wp,
            tc.tile_pool(name="cst", bufs=1) as cst,
            tc.tile_pool(name="ypf", bufs=1) as ypfp,
            tc.tile_pool(name="chain", bufs=2) as chainp,
            tc.tile_pool(name="xt", bufs=2) as xtp,
            tc.tile_pool(name="wk", bufs=1) as wkp,
            tc.tile_pool(name="ps", bufs=1, space="PSUM") as psp,
        ):
            # ---- resident weights & constants ----
            wout = wp.tile([128, 8 * C], bf16, name="wout")
            nc.sync.dma_start(wout[:], wout_d[:])
            wt = {}
            for n in wnames:
                wt[n] = wp.tile([128, 4 * C], bf16, tag=n, name=n)
                nc.sync.dma_start(wt[n][:], wdram[n][:])
            u_t, eu_t, dec_t = {}, {}, {}
            for cb in range(CB):
                sl = slice(cb * 128, (cb + 1) * 128)
                for d, ud, eud, dd in (("f", u_f_d, eu_f_d, dec_f_d),
                                       ("b", u_b_d, eu_b_d, dec_b_d)):
                    u_t[(d, cb)] = cst.tile([128, 1], f32, tag=f"u{d}{cb}",
                                            name=f"u{d}{cb}")
                    nc.sync.dma_start(u_t[(d, cb)][:], ud[sl, :])
                    eu_t[(d, cb)] = cst.tile([128, 1], f32, tag=f"e{d}{cb}",
                                             name=f"e{d}{cb}")
                    nc.sync.dma_start(eu_t[(d, cb)][:], eud[sl, :])
                    dec_t[(d, cb)] = cst.tile([128, 1], f32, tag=f"d{d}{cb}",
                                              name=f"d{d}{cb}")
                    nc.sync.dma_start(dec_t[(d, cb)][:], dd[sl, :])

            # forward-direction y, resident in SBUF across both phases
            ypf = {}
            for cb in range(CB):
                ypf[cb] = ypfp.tile([128, T], bf16, tag=f"ypf{cb}",
                                    name=f"ypf{cb}")

            def run_phase(d):
                fwd = d == "f"
                wr, wk, wv = wt["w_r" + d], wt["w_k" + d], wt["w_v" + d]
                pairs = list(range(NP)) if fwd else list(reversed(range(NP)))
                chains = {}

                def chain_buf(cb, kind):
                    key = (cb, kind)
                    t = chainp.tile([128, PW + 1], bf16,
                                    tag=f"ch_{kind}{cb}",
                                    name=f"ch_{kind}{cb}")
                    prev = chains.get(key)
                    chains[key] = t
                    eng = num_eng if kind == "num" else nc.vector
                    if fwd:
                        if prev is None:
                            eng.memset(t[:, 0:1], 0.0)
                        else:
                            eng.tensor_copy(t[:, 0:1], prev[:, PW: PW + 1])
                    else:
                        if prev is None:
                            eng.memset(t[:, PW: PW + 1], 0.0)
                        else:
                            eng.tensor_copy(t[:, PW: PW + 1], prev[:, 0:1])
                    return t

                stash = {}     # (pr, cb) -> (dn, sgm)
                ypb_tiles = {}  # (pr, cb) -> y tile (bwd only)

                def part_a(pr):
                    p0 = pr * PW
                    xts = {}
                    for half, tt in enumerate((2 * pr, 2 * pr + 1)):
                        t0 = tt * TT
                        for kb in range(4):
                            xt = xtp.tile([128, TT], bf16,
                                          tag=f"xt{kb}h{half}",
                                          bufs=2, name=f"xt{kb}h{half}")
                            nc.sync.dma_start(
                                xt[:],
                                xT[kb * 128:(kb + 1) * 128, t0: t0 + TT])
                            xts[(half, kb)] = xt
                    for cb in range(CB):
                        pss = {}
                        for cls, w in (("k", wk), ("v", wv), ("r", wr)):
                            for half in range(2):
                                pss[(cls, half)] = psp.tile(
                                    [128, TT], f32, tag="ps", bufs=8,
                                    name=f"ps{cls}")
                            for kb in range(4):
                                wsl = w[:, kb * C + cb * 128:
                                        kb * C + cb * 128 + 128]
                                for half in range(2):
                                    nc.tensor.matmul(
                                        pss[(cls, half)][:], wsl,
                                        xts[(half, kb)][:],
                                        start=(kb == 0), stop=(kb == 3))
                        # 1024-wide work tiles, written per half
                        ek = wkp.tile([128, PW], bf16, tag="ek", bufs=2,
                                      name="ek")
                        ekb = wkp.tile([128, PW], bf16, tag="ekb", bufs=2,
                                       name="ekb")
                        em = wkp.tile([128, PW], bf16, tag="em", bufs=2,
                                      name="em")
                        ekv = wkp.tile([128, PW], bf16, tag="ekv", bufs=2,
                                       name="ekv")
                        for half in range(2):
                            hs = slice(half * TT, (half + 1) * TT)
                            act(ek[:, hs], pss[("k", half)][:], Act.Exp)
                            act(em[:, hs], pss[("r", half)][:], Act.Exp,
                                bias=0.0, scale=-1.0)
                            nc.vector.tensor_mul(ekv[:, hs], ek[:, hs],
                                                 pss[("v", half)][:])
                        lt = wkp.tile([128, PW], f32, tag="lt", bufs=2,
                                      name="lt")
                        act(lt[:], em[:], Act.Ln, bias=1.0)
                        sg = wkp.tile([128, PW], bf16, tag="sg", bufs=4,
                                      name="sg")
                        act(sg[:], lt[:], Act.Exp, scale=-1.0)
                        nc.vector.tensor_scalar_mul(
                            ekb[:], ek[:], eu_t[(d, cb)][:, 0:1])
                        # scans on DVE (full pair width)
                        decbc = dec_t[(d, cb)][:, 0:1].broadcast_to([128, PW])
                        denb = chain_buf(cb, "den")
                        numb = chain_buf(cb, "num")
                        if fwd:
                            nc.vector.tensor_tensor_scan(
                                denb[:, 1: 1 + PW], decbc, ek[:],
                                denb[:, 0:1], Alu.mult, Alu.add)
                            num_eng.tensor_tensor_scan(
                                numb[:, 1: 1 + PW], decbc, ekv[:],
                                numb[:, 0:1], Alu.mult, Alu.add)
                            den_prev = denb[:, 0:PW]
                            num_prev = numb[:, 0:PW]
                        else:
                            nc.vector.tensor_tensor_scan(
                                denb[:, 0:PW][:, ::-1], decbc,
                                ek[:][:, ::-1], denb[:, PW: PW + 1],
                                Alu.mult, Alu.add)
                            num_eng.tensor_tensor_scan(
                                numb[:, 0:PW][:, ::-1], decbc,
                                ekv[:][:, ::-1], numb[:, PW: PW + 1],
                                Alu.mult, Alu.add)
                            den_prev = denb[:, 1: 1 + PW]
                            num_prev = numb[:, 1: 1 + PW]
                        dn = wkp.tile([128, PW], bf16, tag="dn", bufs=4,
                                      name="dn")
                        ekbv = wkp.tile([128, PW], bf16, tag="ekbv", bufs=2,
                                        name="ekbv")
                        nm = wkp.tile([128, PW], bf16, tag="nm", bufs=4,
                                      name="nm")
                        nc.vector.tensor_add(dn[:], ekb[:], den_prev)
                        nc.vector.tensor_scalar_mul(
                            ekbv[:], ekv[:], eu_t[(d, cb)][:, 0:1])
                        nc.gpsimd.tensor_add(nm[:], ekbv[:], num_prev)
                        stash[(pr, cb)] = (dn, nm, sg)

                def part_b(pr):
                    p0 = pr * PW
                    for cb in range(CB):
                        dn, nm, sg = stash.pop((pr, cb))
                        ln = wkp.tile([128, PW], f32, tag="ln", bufs=2,
                                      name="ln")
                        act(ln[:], dn[:], Act.Ln)
                        rc = wkp.tile([128, PW], bf16, tag="rc", bufs=2,
                                      name="rc")
                        act(rc[:], ln[:], Act.Exp, scale=-1.0)
                        y1 = wkp.tile([128, PW], bf16, tag="y1", bufs=2,
                                      name="y1")
                        nc.vector.tensor_mul(y1[:], nm[:], rc[:])
                        if fwd:
                            nc.vector.tensor_mul(
                                ypf[cb][:, p0: p0 + PW], y1[:], sg[:])
                        else:
                            yb = wkp.tile([128, PW], bf16, tag=f"ypb{cb}",
                                          bufs=2, name=f"ypb{cb}")
                            nc.vector.tensor_mul(yb[:], y1[:], sg[:])
                            ypb_tiles[(pr, cb)] = yb

                def part_c(pr):
                    p0 = pr * PW
                    for m in range(PW // 128):
                        t0 = p0 + m * 128
                        pso = psp.tile([128, C], f32, tag="ps", bufs=8,
                                       name="pso")
                        for cb in range(CB):
                            nc.tensor.matmul(
                                pso[:],
                                ypf[cb][:, t0: t0 + 128],
                                wout[:, cb * C: (cb + 1) * C],
                                start=(cb == 0), stop=False)
                        for cb in range(CB):
                            nc.tensor.matmul(
                                pso[:],
                                ypb_tiles[(pr, cb)][:, m * 128:
                                                    (m + 1) * 128],
                                wout[:, (4 + cb) * C: (5 + cb) * C],
                                start=False, stop=(cb == 3))
                        osb = wkp.tile([128, C], bf16, tag="osb",
                                       bufs=2, name="osb")
                        act_copy(osb[:], pso[:])
                        nc.sync.dma_start(out_d[t0: t0 + 128, :], osb[:])

                # group pairs in twos: A A B B (C C) to batch the
                # Reciprocal table epoch across 2 pairs
                for pr in pairs:
                    part_a(pr)
                    part_b(pr)
                    if not fwd:
                        part_c(pr)
                        for key in list(ypb_tiles):
                            if key[0] == pr:
                                del ypb_tiles[key]

            run_phase("f")
            run_phase("b")

    return nc


def _host_prep(x, W_rkv, W_out, time_decay, time_first, time_decay_rev,
               time_first_rev):
    bf16 = ml_dtypes.bfloat16
    f32 = np.float32

    Wr = W_rkv.reshape(C, 2, 3, C)
    pieces = {
        "w_rf": Wr[:, 0, 0], "w_kf": Wr[:, 0, 1], "w_vf": Wr[:, 0, 2],
        "w_rb": Wr[:, 1, 0], "w_kb": Wr[:, 1, 1], "w_vb": Wr[:, 1, 2],
    }
    wmaps = {}
    for n, p in pieces.items():
        wmaps[n] = np.ascontiguousarray(
            p.reshape(4, 128, C).transpose(1, 0, 2).reshape(128, 4 * C)
        ).astype(bf16)

    Wo = W_out.reshape(8, 128, C).transpose(1, 0, 2).reshape(128, 8 * C)
    wout = np.ascontiguousarray(Wo).astype(bf16)

    u_f = np.ascontiguousarray(time_first.reshape(C, 1)).astype(f32)
    u_b = np.ascontiguousarray(time_first_rev.reshape(C, 1)).astype(f32)
    eu_f = np.exp(time_first.astype(np.float64)).reshape(C, 1).astype(f32)
    eu_b = np.exp(time_first_rev.astype(np.float64)).reshape(C, 1).astype(f32)
    dec_f = np.exp(-np.exp(time_decay.astype(np.float64))).reshape(C, 1).astype(f32)
    dec_b = np.exp(-np.exp(time_decay_rev.astype(np.float64))).reshape(C, 1).astype(f32)

    shared = dict(wout=wout, u_f=u_f, u_b=u_b, eu_f=eu_f, eu_b=eu_b,
                  dec_f=dec_f, dec_b=dec_b, **wmaps)
    in_maps = []
    for b in range(B):
        m = dict(shared)
        m["xT"] = np.ascontiguousarray(x[b].T).astype(bf16)
        in_maps.append(m)
    return in_maps


def kernel(x, W_rkv, W_out, time_decay, time_first, time_decay_rev,
           time_first_rev, _trace=False):
    from concourse.bass_utils import run_bass_kernel_spmd

    x = np.asarray(x, dtype=np.float32)
    W_rkv = np.asarray(W_rkv, dtype=np.float32)
    W_out = np.asarray(W_out, dtype=np.float32)
    time_decay = np.asarray(time_decay, dtype=np.float32)
    time_first = np.asarray(time_first, dtype=np.float32)
    time_decay_rev = np.asarray(time_decay_rev, dtype=np.float32)
    time_first_rev = np.asarray(time_first_rev, dtype=np.float32)

    if "nc" not in _CACHE:
        _CACHE["nc"] = _build_nc()
    nc = _CACHE["nc"]

    in_maps = _host_prep(x, W_rkv, W_out, time_decay, time_first,
                         time_decay_rev, time_first_rev)
    res = run_bass_kernel_spmd(
        nc, in_maps, core_ids=list(range(B)), trace=_trace
    )
    _CACHE["last_result"] = res
    out = np.stack([res.results[b]["y"].astype(np.float32) for b in range(B)])
    return out
